# revision 16
# baseline (speedup 1.0000x reference)
"""DRN layer kernel for 8 TRN2 NeuronCores.

Math (reference):
    T[j,k,l,m]   = exp(-w[j,k] * (s0[m]-s1[l])^2)
    Pw[i,j,k,l]  = sum_m T[j,k,l,m] * P[i,k,m]
    logsum[i,j,l]= sum_k log(Pw[i,j,k,l])
    out          = softmax_l(logsum + exponent_B[j,l])

Sharding: tensor-parallel over n_upper (j): 8 cores x 8 upper nodes each,
every core sees the full batch. T depends only on the weights, so it is
precomputed on the host and shipped per-core in matmul-ready layout.

Device math uses a bf16 matmul with an exactness trick: T = 1 + t with
|t| <= 0.11, so Pw = S + sum_m t*P where S = sum_m P. t and P are sent
as bf16 (quantization error scales with |t|, not |T|~1), and S rides in
the matmul as two extra contraction rows (hi/lo bf16 split) against
columns of ones, so PSUM receives near-fp32-accurate Pw at bf16 speed.

Per k-pair the two PSUM tiles are multiplied on VectorE (halving the
log count: log(a*b) = log a + log b), logged on ScalarE in 2048-wide
calls, and the running (j,l)-sum is accumulated on GpSimd + VectorE.
Softmax over l finishes on VectorE/ScalarE.
"""

import numpy as np

B, NU, NL, QU, QL = 256, 64, 64, 64, 64
NCORES = 8
JLOC = NU // NCORES  # 8 upper nodes per core
JL = JLOC * QU       # 512 = packed (j, l) free dim
KDIM = QL + 2        # 66 = contraction: 64 m-rows + S_hi + S_lo rows
PW = B + JL          # 768 packed free width of PTT

NPAIR = NL // 2      # 32 k-pairs
NGRP = 8             # pair groups per ih; each group = 4 pairs = 8 k


def _build_program():
    import concourse.bass as bass
    import concourse.bacc as bacc
    import concourse.mybir as mybir
    from concourse.tile import TileContext

    f32 = mybir.dt.float32
    bf16 = mybir.dt.bfloat16
    AF = mybir.ActivationFunctionType

    nc = bacc.Bacc(None, target_bir_lowering=False)
    PTT = nc.declare_dram_parameter("PTT", [NL, KDIM, PW], bf16, isOutput=False)
    EB = nc.declare_dram_parameter("EB", [128, JL], f32, isOutput=False)
    OUT = nc.declare_dram_parameter("out", [2, 128, JL], f32, isOutput=True)

    # 32 super-groups (sg) of 2 k's each; PSUM tile [128, 2048] holds the 4
    # Pw quarters [ih0k0 | ih1k0 | ih0k1 | ih1k1]. Route A sgs are drained
    # by ScalarE (Ln straight from PSUM) + GpSimd/VectorE adds; route B sgs
    # feed VectorE running-product chains (one PSUM operand per op), whose
    # chunk-logs land on ScalarE every ~10 sgs.
    NSG = NL // 2
    A_SET = {sg for sg in range(NSG) if (sg * 13) % NSG < 13}
    b_list = [sg for sg in range(NSG) if sg not in A_SET]
    CHUNK = (len(b_list) + 1) // 2  # 2 chunks of product depth <= 10
    b_chunk = {sg: i // CHUNK for i, sg in enumerate(b_list)}
    b_pos = {sg: i % CHUNK for i, sg in enumerate(b_list)}
    b_last = {sg: (i % CHUNK == CHUNK - 1) or (i == len(b_list) - 1)
              for i, sg in enumerate(b_list)}

    with TileContext(nc) as tc:
        with (
            tc.tile_pool(name="ptt", bufs=8) as ppool,
            tc.tile_pool(name="eb", bufs=1) as ebpool,
            tc.tile_pool(name="ps", bufs=2, space="PSUM") as pspool,
            tc.tile_pool(name="lg", bufs=4) as lgpool,
            tc.tile_pool(name="pb", bufs=1) as pbpool,
            tc.tile_pool(name="acc", bufs=1) as apool,
            tc.tile_pool(name="sm", bufs=4) as smpool,
            tc.tile_pool(name="ot", bufs=4) as opool,
        ):
            ebt = ebpool.tile([128, JL], f32, tag="ebt")
            nc.sync.dma_start(out=ebt[:], in_=EB[:, :])
            acc = [apool.tile([128, JL], f32, tag=f"acc{ih}", name=f"acc{ih}")
                   for ih in range(2)]
            acc_started = [False, False]
            add_rr = 0

            def accumulate(sl, ih, eng):
                if not acc_started[ih]:
                    # first accumulate folds in exponent_B
                    eng.tensor_add(acc[ih][:], ebt[:], sl)
                    acc_started[ih] = True
                else:
                    eng.tensor_add(acc[ih][:], acc[ih][:], sl)

            prodbuf = [None, None]
            for sg in range(NSG):
                ps = pspool.tile([128, 4 * JL], f32, tag="ps", name="ps")
                for kk in range(2):
                    k = 2 * sg + kk
                    ptt = ppool.tile([KDIM, PW], bf16, tag="ptt")
                    nc.sync.dma_start(out=ptt[:], in_=PTT[k])
                    for ih in range(2):
                        col = (kk * 2 + ih) * JL
                        nc.tensor.matmul(
                            ps[:, col:col + JL],
                            lhsT=ptt[:, ih * 128:(ih + 1) * 128],
                            rhs=ptt[:, B:],
                            start=True,
                            stop=True,
                        )
                if sg in A_SET:
                    lg = lgpool.tile([128, 4 * JL], f32, tag="lg", name="lg")
                    nc.scalar.activation(lg[:], ps[:], AF.Ln)
                    for q in range(4):
                        ih = q % 2
                        sl = lg[:, q * JL:(q + 1) * JL]
                        add_rr += 1
                        eng = nc.vector if add_rr % 8 == 0 else nc.gpsimd
                        accumulate(sl, ih, eng)
                else:
                    ci = b_chunk[sg]
                    if b_pos[sg] == 0:
                        prodbuf[ci] = pbpool.tile([128, 4 * JL], f32,
                                                  tag=f"pb{ci}",
                                                  name=f"pb{ci}")
                        nc.vector.tensor_copy(prodbuf[ci][:], ps[:])
                    else:
                        nc.vector.tensor_mul(
                            prodbuf[ci][:], ps[:], prodbuf[ci][:])
                    if b_last[sg]:
                        clg = lgpool.tile([128, 4 * JL], f32, tag="lg",
                                          name="clg")
                        nc.scalar.activation(clg[:], prodbuf[ci][:], AF.Ln)
                        for q in range(4):
                            ih = q % 2
                            sl = clg[:, q * JL:(q + 1) * JL]
                            accumulate(sl, ih, nc.gpsimd)

            for ih in range(2):
                for j in range(JLOC):
                    seg = acc[ih][:, j * QU:(j + 1) * QU]
                    negmx = smpool.tile([128, 1], f32, tag="negmx")
                    nc.vector.tensor_reduce(
                        negmx[:], seg, axis=mybir.AxisListType.X,
                        op=mybir.AluOpType.max, negate=True,
                    )
                    ex = opool.tile([128, QU], f32, tag="ex")
                    sm = smpool.tile([128, 1], f32, tag="sm")
                    nc.scalar.activation(
                        ex[:], seg, AF.Exp, bias=negmx[:], accum_out=sm[:],
                    )
                    rc = smpool.tile([128, 1], f32, tag="rc")
                    nc.vector.reciprocal(rc[:], sm[:])
                    ot = opool.tile([128, QU], f32, tag="ot")
                    nc.vector.tensor_scalar_mul(ot[:], ex[:], rc[:])
                    nc.sync.dma_start(
                        out=OUT[ih, :, j * QU:(j + 1) * QU], in_=ot[:],
                    )
    nc.compile()
    return nc


def _host_prep(P, weight, bias_abs, bias_q, lambda_abs, lambda_q):
    """Build per-core input maps. Host-side, cheap (T is ~64MB total)."""
    import ml_dtypes

    bf16 = ml_dtypes.bfloat16
    s1 = (np.arange(QU, dtype=np.float64) / QU)
    s0 = (np.arange(QL, dtype=np.float64) / QL)
    diff2 = (s0[None, :] - s1[:, None]) ** 2             # [l, m]
    # t[j, k, l, m] = T - 1, in bf16 (error scales with |t| <= 0.11)
    t_full = np.expm1(-weight[:, :, None, None].astype(np.float64)
                      * diff2[None, None, :, :]).astype(np.float32)
    sq = s1
    expB = (-bias_q.astype(np.float64) * (sq[None, :] - lambda_q) ** 2
            - bias_abs.astype(np.float64)
            * np.abs(sq[None, :] - lambda_abs)).astype(np.float32)

    P32 = P.astype(np.float32)
    S = P32.sum(axis=2, dtype=np.float64).astype(np.float32)   # [i, k]
    S_hi = S.astype(bf16)
    S_lo = (S - S_hi.astype(np.float32)).astype(bf16)

    PT_bf = P32.transpose(1, 2, 0).astype(bf16)          # [k, m, i]

    in_maps = []
    for c in range(NCORES):
        tc_ = t_full[c * JLOC:(c + 1) * JLOC]            # [8, k, l, m]
        tc_ = tc_.transpose(1, 3, 0, 2).reshape(NL, QL, JL)  # [k, m, (j,l)]
        PTTc = np.empty((NL, KDIM, PW), dtype=bf16)
        PTTc[:, :QL, :B] = PT_bf
        PTTc[:, QL, :B] = S_hi.T                         # row 64: S_hi
        PTTc[:, QL + 1, :B] = S_lo.T                     # row 65: S_lo
        PTTc[:, :QL, B:] = tc_.astype(bf16)
        PTTc[:, QL:, B:] = bf16(1.0)                     # ones against S rows
        EBc = np.ascontiguousarray(np.broadcast_to(
            expB[c * JLOC:(c + 1) * JLOC].reshape(1, JL), (128, JL)))
        in_maps.append({"PTT": PTTc, "EB": EBc})
    return in_maps


_PROGRAM = None


def _get_program():
    global _PROGRAM
    if _PROGRAM is None:
        _PROGRAM = _build_program()
    return _PROGRAM


def run_on_device(in_maps, trace=False):
    from concourse.bass_utils import run_bass_kernel_spmd
    nc = _get_program()
    return run_bass_kernel_spmd(
        nc, in_maps, core_ids=list(range(NCORES)), trace=trace,
    )


def assemble(results):
    out = np.empty((B, NU, QU), dtype=np.float32)
    for c in range(NCORES):
        rc = results[c]["out"].reshape(B, JLOC, QU)
        out[:, c * JLOC:(c + 1) * JLOC, :] = rc
    return out


def kernel(P, weight, bias_abs, bias_q, lambda_abs, lambda_q):
    in_maps = _host_prep(P, weight, bias_abs, bias_q, lambda_abs, lambda_q)
    res = run_on_device(in_maps, trace=False)
    return assemble(res.results)
